# revision 22
# baseline (speedup 1.0000x reference)
"""AttentionBlock3D Trainium2 kernel.

Module: GroupNorm(8 groups) -> 1x1x1 conv QKV -> 4-head attention over
N=4096 spatial positions (head_dim 64) -> 1x1x1 conv proj -> residual.
Shapes: x [2, 256, 16, 16, 16] f32.

Sharding (8 cores): batch (2) x query-range (4 chunks of 1024 rows).
Each core computes, for its batch b and query rows nq:
  - GroupNorm stats over the full x[b] (redundant per-batch, cheap),
    folded into a per-channel affine (s_c, t_c) applied on the fly.
  - k, v for ALL 4096 keys (all heads), q only for its 1024 rows.
  - full attention for all 4 heads on its rows; softmax is computed
    unnormalized (exp, no max subtraction -- scores are O(1) here) with
    the denominator obtained by an ones-matmul on the PE, and the
    normalization folded in after the attention*V matmul.
  - proj + bias + residual for its [256, 1024] output slice.
Gather on host is pure concatenation.

Layouts on device (per core):
  x  [C=256, N]   -> 2 channel-tiles of [128, N] (channels on partitions)
  k_sb[pair]      [128, 4096] bf16: partitions = [head 2p (64) ; head 2p+1]
  q_sb[pair]      [128, 1024] bf16: same head-pair packing
  vT_sb           [128, 32*256] bf16: partitions = key-tile rows m,
                  free = m-tile * 256 + v-channel
  scores^T        PSUM [m 128, n 512] via row-tiled (K=64) matmul pairs
  attention out   o[c 128, n 512] via col-tiled (M=64) matmul pairs
Softmax denominator: ones[128,64] lhsT col-tiled -> sg[128,512] where
rows 0:64 = sigma(head even) replicated, 64:128 = sigma(head odd).
"""

import numpy as np

B = 2
C = 256
NH = 4
GROUPS = 8
EPS = 1e-5
N = 16 * 16 * 16  # 4096
HD = C // NH      # 64
NQ = N // 4       # 1024 query rows per core
NCORES = 8
CT = 2            # channel tiles of 128
MT = N // 128     # 32 key tiles
SCALE = HD ** -0.5
# Schraudolph bf16 exp: bits_i16(round(A*x + B)) viewed as bf16 ~= exp(x).
# A folds the attention scale; c tuned for min max-rel-err (~3.3%), which
# is benign here (scores span +-2.5, softmax-normalized, verified 1.3e-4
# end-to-end on this problem's data).
import math
EXP_A = SCALE * 128.0 / math.log(2.0)
EXP_B = 127.0 * 128.0 - 0.0430 * 128.0
# fp8e4m3 variant (int8 bits): [s][4 exp][3 mant]
EXP8_A = SCALE * 8.0 / math.log(2.0)
EXP8_B = 7.0 * 8.0 - 0.0430 * 8.0


def _build_nc(finalize=True):
    import concourse.bacc as bacc
    import concourse.bass as bass
    import concourse.mybir as mybir
    from concourse.tile import TileContext

    f32 = mybir.dt.float32
    bf16 = mybir.dt.bfloat16
    Alu = mybir.AluOpType
    AF = mybir.ActivationFunctionType

    nc = bacc.Bacc("TRN2", debug=False)

    xf = nc.dram_tensor("xf", [C, N], f32, kind="ExternalInput").ap()
    xq = nc.dram_tensor("xq", [C, NQ], f32, kind="ExternalInput").ap()
    wT = nc.dram_tensor("wT", [C, 3 * C], bf16, kind="ExternalInput").ap()
    pT = nc.dram_tensor("pT", [C, C], bf16, kind="ExternalInput").ap()
    qb = nc.dram_tensor("qb", [3 * C], f32, kind="ExternalInput").ap()
    pb = nc.dram_tensor("pb", [C], f32, kind="ExternalInput").ap()
    nw = nc.dram_tensor("nw", [C], f32, kind="ExternalInput").ap()
    nb = nc.dram_tensor("nb", [C], f32, kind="ExternalInput").ap()
    Gm = nc.dram_tensor("Gm", [CT, 128, GROUPS], f32, kind="ExternalInput").ap()
    Pm = nc.dram_tensor("Pm", [CT, GROUPS, 128], f32, kind="ExternalInput").ap()
    y = nc.dram_tensor("y", [C, NQ], f32, kind="ExternalOutput").ap()

    def bcast_ap(src_1d, parts):
        # view a 1-D dram AP as [parts, len] with 0-stride partition dim
        return bass.AP(
            tensor=src_1d.tensor,
            offset=src_1d.offset,
            ap=[[0, parts]] + list(src_1d.ap),
        )

    with TileContext(nc) as tc:
        import contextlib

        est = contextlib.ExitStack()
        with est:
            singles = est.enter_context(tc.tile_pool(name="singles", bufs=1))
            work = est.enter_context(tc.tile_pool(name="work", bufs=2))

            # ---------- persistent SBUF tiles ----------
            x_sb = [singles.tile([128, N], f32, tag=f"x{ct}", name=f"x{ct}") for ct in range(CT)]
            xq_sb = [singles.tile([128, NQ], f32, tag=f"xq{ct}", name=f"xq{ct}") for ct in range(CT)]
            xn_sb = [singles.tile([128, N], bf16, tag=f"xn{ct}", name=f"xn{ct}") for ct in range(CT)]
            xqn_sb = [singles.tile([128, NQ], bf16, tag=f"xqn{ct}", name=f"xqn{ct}") for ct in range(CT)]
            wT_sb = [singles.tile([128, 3 * C], bf16, tag=f"wT{ct}", name=f"wT{ct}") for ct in range(CT)]
            pT_sb = [singles.tile([128, C], bf16, tag=f"pT{ct}", name=f"pT{ct}") for ct in range(CT)]
            k_sb = [singles.tile([128, N], bf16, tag=f"k{p}", name=f"k{p}") for p in range(2)]
            q_sb = [singles.tile([128, NQ], bf16, tag=f"q{p}", name=f"q{p}") for p in range(2)]
            # vT8: fp8e4m3 AV stationary operand, DoubleRow layout.
            # [p, mtp(16):1024, hp(2):512, hh(2):256, j(2):128, d(128):1]
            # where d = [v(64)|ones(64)] for hh=0, [ones(64)|v(64)] for hh=1;
            # contraction key = 128*(2*mtp + j) + p.
            f8 = mybir.dt.float8e4
            i8 = mybir.dt.int8
            vT8_sb = singles.tile([128, MT * 512], f8, tag="vT8", name="vT8")
            qb_sb = singles.tile([128, 6], f32, tag="qb", name="qb")
            pb_sb = singles.tile([128, 2], f32, tag="pb", name="pb")
            nw_sb = singles.tile([128, CT], f32, tag="nw", name="nw")
            nb_sb = singles.tile([128, CT], f32, tag="nb", name="nb")
            G_sb = [singles.tile([128, GROUPS], f32, tag=f"G{ct}", name=f"G{ct}") for ct in range(CT)]
            P_sb = [singles.tile([GROUPS, 128], f32, tag=f"P{ct}", name=f"P{ct}") for ct in range(CT)]
            eps_sb = singles.tile([128, 1], f32, tag="eps", name="eps")
            s_sb = [singles.tile([128, 1], f32, tag=f"s{ct}", name=f"s{ct}") for ct in range(CT)]
            t_sb = [singles.tile([128, 1], f32, tag=f"t{ct}", name=f"t{ct}") for ct in range(CT)]
            mr_sb = singles.tile([GROUPS, 2], f32, tag="mr", name="mr")

            # ---------- input DMA ----------
            for ct in range(CT):
                cs = slice(128 * ct, 128 * (ct + 1))
                for dc in range(4):
                    ds_ = slice(1024 * dc, 1024 * (dc + 1))
                    nc.sync.dma_start(out=x_sb[ct][:, ds_], in_=xf[cs, ds_])
                nc.sync.dma_start(out=xq_sb[ct], in_=xq[cs, :])
                nc.sync.dma_start(out=wT_sb[ct], in_=wT[cs, :])
                nc.sync.dma_start(out=pT_sb[ct], in_=pT[cs, :])
                nc.sync.dma_start(out=G_sb[ct], in_=Gm[ct])
                nc.sync.dma_start(out=P_sb[ct], in_=Pm[ct])
            nc.sync.dma_start(out=qb_sb, in_=qb.rearrange("(t p) -> p t", p=128))
            nc.sync.dma_start(out=pb_sb, in_=pb.rearrange("(t p) -> p t", p=128))
            nc.sync.dma_start(out=nw_sb, in_=nw.rearrange("(t p) -> p t", p=128))
            nc.sync.dma_start(out=nb_sb, in_=nb.rearrange("(t p) -> p t", p=128))
            nc.vector.memset(eps_sb, EPS)

            # ---------- GroupNorm statistics ----------
            with tc.tile_pool(name="ph1psum", bufs=1, space="PSUM") as pp, \
                 tc.tile_pool(name="warm", bufs=1, space="PSUM") as wmp, \
                 tc.tile_pool(name="stats", bufs=2) as stp:
                # modest PE pre-warm while the stats chain runs, so the QKV
                # matmuls start at 2.4 GHz instead of the throttled clock
                warm_ps = wmp.tile([128, 512], f32, name="warm_ps")
                for _ in range(18):
                    nc.tensor.matmul(
                        warm_ps,
                        x_sb[0][:, 0:128],
                        x_sb[0][:, 0:512],
                        start=True, stop=True,
                    )
                gs_ps = pp.tile([GROUPS, 2], f32, tag="gs", name="gs")
                NSUB = N // 512
                for ct in range(CT):
                    stats = stp.tile([128, NSUB, 6], f32, tag="bnst", name="bnst")
                    for i in range(NSUB):
                        nc.vector.bn_stats(
                            out=stats[:, i, :], in_=x_sb[ct][:, 512 * i: 512 * (i + 1)]
                        )
                    mv = stp.tile([128, 2], f32, tag="mv", name="mv")
                    nc.vector.bn_aggr(out=mv, in_=stats)
                    # cstat = [mean_c, E[x^2]_c]
                    cstat = stp.tile([128, 2], f32, tag="cstat", name="cstat")
                    nc.vector.tensor_copy(out=cstat[:, 0:1], in_=mv[:, 0:1])
                    m2 = stp.tile([128, 1], f32, tag="m2", name="m2")
                    nc.vector.tensor_mul(out=m2, in0=mv[:, 0:1], in1=mv[:, 0:1])
                    nc.vector.tensor_add(out=cstat[:, 1:2], in0=mv[:, 1:2], in1=m2)
                    # group sums: gs[g, :] = sum_c G[c, g] * cstat[c, :] / (fold 1/32 in G)
                    nc.tensor.matmul(
                        gs_ps, G_sb[ct], cstat, start=(ct == 0), stop=(ct == CT - 1)
                    )
                # mr = [mean_g, rstd_g]
                nc.vector.tensor_copy(out=mr_sb[:, 0:1], in_=gs_ps[:, 0:1])
                gm2 = stp.tile([GROUPS, 1], f32, tag="gm2", name="gm2")
                # only one DVE input may come from PSUM -> square the SBUF copy
                nc.vector.tensor_mul(out=gm2, in0=mr_sb[:, 0:1], in1=mr_sb[:, 0:1])
                var_g = stp.tile([GROUPS, 1], f32, tag="varg", name="varg")
                nc.vector.tensor_sub(out=var_g, in0=gs_ps[:, 1:2], in1=gm2)
                sd_g = stp.tile([GROUPS, 1], f32, tag="sdg", name="sdg")
                nc.scalar.activation(
                    out=sd_g, in_=var_g, func=AF.Sqrt, bias=eps_sb[0:GROUPS, :],
                )
                nc.vector.reciprocal(out=mr_sb[:, 1:2], in_=sd_g)
                # broadcast to channels, build per-channel affine s, t
                for ct in range(CT):
                    pc_ps = pp.tile([128, 2], f32, tag="pc", name="pc")
                    nc.tensor.matmul(pc_ps, P_sb[ct], mr_sb, start=True, stop=True)
                    nc.vector.tensor_mul(
                        out=s_sb[ct], in0=pc_ps[:, 1:2], in1=nw_sb[:, ct: ct + 1]
                    )
                    tt = stp.tile([128, 1], f32, tag="tt", name="tt")
                    nc.vector.tensor_mul(out=tt, in0=pc_ps[:, 0:1], in1=s_sb[ct])
                    nc.vector.tensor_sub(
                        out=t_sb[ct], in0=nb_sb[:, ct: ct + 1], in1=tt
                    )
                # normalized inputs (bf16): xn = x * s + t  (on ACT, per-partition affine)
                for ct in range(CT):
                    for dc in range(4):
                        ds_ = slice(1024 * dc, 1024 * (dc + 1))
                        nc.vector.tensor_scalar(
                            out=xn_sb[ct][:, ds_], in0=x_sb[ct][:, ds_],
                            scalar1=s_sb[ct], scalar2=t_sb[ct],
                            op0=Alu.mult, op1=Alu.add,
                        )
                    nc.vector.tensor_scalar(
                        out=xqn_sb[ct], in0=xq_sb[ct], scalar1=s_sb[ct],
                        scalar2=t_sb[ct], op0=Alu.mult, op1=Alu.add,
                    )

            # ---------- QKV projections ----------
            # vT8 holds fp8 v (no bias: it is folded into the proj bias on
            # the host, valid because the attention weights sum to 1) plus
            # ones columns that make each AV matmul also produce the softmax
            # denominator in the complementary partition rows of the same
            # accumulator bank, for free.
            def v8_view(off, dims):
                return bass.AP(
                    tensor=vT8_sb.tensor,
                    offset=vT8_sb.offset + off,
                    ap=[list(vT8_sb.ap[0])] + [list(d) for d in dims],
                )

            # ones blocks: hh=0 -> d 64:128, hh=1 -> d 0:64 (offset 64+192*hh)
            for hh in range(2):
                nc.gpsimd.memset(
                    v8_view(64 + 192 * hh,
                            [[1024, 16], [512, 2], [128, 2], [1, 64]]),
                    1.0,
                )
            with tc.tile_pool(name="qkvpsum", bufs=3, space="PSUM") as qp, \
                 tc.tile_pool(name="vtpsum", bufs=3, space="PSUM") as vp:
                # q first: unblocks the first attention iteration earliest
                for hp in range(2):
                    for chk in range(NQ // 512):
                        ns = slice(512 * chk, 512 * (chk + 1))
                        qps = qp.tile([128, 512], f32, tag="kq", name="kq")
                        for ct in range(CT):
                            nc.tensor.matmul(
                                qps,
                                wT_sb[ct][:, 128 * hp: 128 * (hp + 1)],
                                xqn_sb[ct][:, ns],
                                start=(ct == 0), stop=(ct == CT - 1),
                            )
                        nc.scalar.activation(
                            out=q_sb[hp][:, ns], in_=qps, func=AF.Identity,
                            bias=qb_sb[:, hp: hp + 1],
                        )
                # k for pair 0, then vT (attention on pair 0 needs both),
                # then k for pair 1.
                def emit_k(hp):
                    for chk in range(N // 512):
                        ns = slice(512 * chk, 512 * (chk + 1))
                        kp = qp.tile([128, 512], f32, tag="kq", name="kq")
                        for ct in range(CT):
                            nc.tensor.matmul(
                                kp,
                                wT_sb[ct][:, C + 128 * hp: C + 128 * (hp + 1)],
                                xn_sb[ct][:, ns],
                                start=(ct == 0), stop=(ct == CT - 1),
                            )
                        nc.scalar.activation(
                            out=k_sb[hp][:, ns], in_=kp, func=AF.Identity,
                            bias=qb_sb[:, 2 + hp: 3 + hp],
                        )

                emit_k(0)
                for mtp in range(MT // 2):
                    vps = vp.tile([128, 2 * C], f32, tag="vt", name="vt")
                    for j in range(2):
                        ms = slice(128 * (2 * mtp + j), 128 * (2 * mtp + j + 1))
                        for ct in range(CT):
                            nc.tensor.matmul(
                                vps[:, C * j: C * (j + 1)],
                                xn_sb[ct][:, ms],
                                wT_sb[ct][:, 2 * C: 3 * C],
                                start=(ct == 0), stop=(ct == CT - 1),
                            )
                    # strided f32->fp8 converts scattering the v channels into
                    # the DoubleRow layout (one per head pair: ISA caps APs at
                    # 3 free dims); v channel c = (hp, hh, dv), d-off 64*hh.
                    src = vps.rearrange(
                        "p (j hp hh dv) -> p hp j hh dv", j=2, hp=2, hh=2
                    )
                    for hp_ in range(2):
                        dst = v8_view(
                            1024 * mtp + 512 * hp_,
                            [[128, 2], [320, 2], [1, 64]],
                        )
                        if (2 * mtp + hp_) % 2 == 0:
                            nc.scalar.copy(out=dst, in_=src[:, hp_])
                        else:
                            nc.vector.tensor_copy(out=dst, in_=src[:, hp_])
                emit_k(1)

            # ---------- attention + proj ----------
            # PSUM budget (8 banks): scores pool [128,1536]x2 = 6 banks,
            # "acc" tag pool [128,512]x2 = 2 banks. Each accumulator bank
            # receives one combined AV+sigma matmul stream (full M=128:
            # 64 v columns + 64 ones columns), so o and its softmax
            # denominator land in complementary partition halves of the
            # same bank. The proj matmuls reuse the acc slots.
            GRPS = [list(range(i, min(i + 2, MT))) for i in range(0, MT, 2)]
            with tc.tile_pool(name="scps", bufs=2, space="PSUM") as scp, \
                 tc.tile_pool(name="accps", bufs=2, space="PSUM") as accp, \
                 tc.tile_pool(name="heat", bufs=1, space="PSUM") as heatp, \
                 tc.tile_pool(name="esb", bufs=3) as esb, \
                 tc.tile_pool(name="osb", bufs=2) as osb, \
                 tc.tile_pool(name="outsb", bufs=2) as outsb:
                # HAM heater: the attention pipeline is exp-paced; without
                # filler matmuls the PE idles enough for the clock gate to
                # drop it to 1.2 GHz, which doubles every matmul.
                heat_ps = heatp.tile([128, 512], f32, name="heat_ps")
                for cn in range(NQ // 512):
                    ns = slice(512 * cn, 512 * (cn + 1))
                    on_sb = [None, None]
                    for hp in range(2):
                        # acc[hh]: even head: [o(0:64); sigma(64:128)]
                        #          odd head:  [sigma(0:64); o(64:128)]
                        acc = [accp.tile([128, 512], f32, tag="acc", name=f"acc{h}")
                               for h in range(2)]
                        for gi, grp in enumerate(GRPS):
                            gl = len(grp)
                            e_h = []
                            for hh in range(2):
                                sc = scp.tile([128, 1024], f32, tag="sc", name="sc")
                                for j, mt in enumerate(grp):
                                    nc.tensor.matmul(
                                        sc[:, 512 * j: 512 * (j + 1)],
                                        k_sb[hp][64 * hh: 64 * (hh + 1),
                                                 128 * mt: 128 * (mt + 1)],
                                        q_sb[hp][64 * hh: 64 * (hh + 1), ns],
                                        start=True, stop=True,
                                        tile_position=(64 * hh, 0),
                                    )
                                e = esb.tile([128, 1024], f8, tag="e", name="e")
                                # ACT does exact exp; DVE does Schraudolph
                                # (int8 bits of fp8e4m3). hh0 -> ACT, hh1 ->
                                # DVE except every 8th group (load balance).
                                on_act = (hh == 0) or (gi % 16 == 15)
                                if on_act:
                                    nc.scalar.activation(
                                        out=e[:, : 512 * gl], in_=sc[:, : 512 * gl],
                                        func=AF.Exp, scale=SCALE,
                                    )
                                else:
                                    nc.vector.tensor_scalar(
                                        out=e.bitcast(i8)[:, : 512 * gl],
                                        in0=sc[:, : 512 * gl],
                                        scalar1=EXP8_A, scalar2=EXP8_B,
                                        op0=Alu.mult, op1=Alu.add,
                                    )
                                e_h.append(e)
                            # AV: one fp8 DoubleRow matmul per head covers the
                            # whole 2-key-tile group (K = 256 as [128, j=2]).
                            first = (gi == 0)
                            last = (gi == len(GRPS) - 1)
                            for hh in range(2):
                                nc.tensor.matmul(
                                    acc[hh],
                                    v8_view(
                                        1024 * gi + 512 * hp + 256 * hh,
                                        [[128, 2], [1, 128]],
                                    ),
                                    e_h[hh].rearrange("p (j n) -> p j n", j=2),
                                    start=first, stop=last,
                                    perf_mode=mybir.MatmulPerfMode.DoubleRow,
                                )
                            for _ in range(2):
                                nc.tensor.matmul(
                                    heat_ps[:, 0:256],
                                    xn_sb[0][:, 0:128],
                                    xn_sb[0][:, 0:256],
                                    start=True, stop=True,
                                )
                        # normalize: sigma sits in the complementary partition
                        # half; reciprocal locally, DMA the [64, 512] block to
                        # the o rows' partitions, multiply.
                        rec = osb.tile([128, 512], f32, tag="rec", name="rec")
                        recb = osb.tile([128, 512], f32, tag="recb", name="recb")
                        rec2 = osb.tile([128, 512], f32, tag="rec2", name="rec2")
                        on = osb.tile([128, 512], bf16, tag="on", name="on")
                        # reciprocal_approx_fast requires base_partition 0, so
                        # run it over the full tile; only the sigma half of
                        # each result is used (the o half is don't-care).
                        # even head: o rows 0:64, sigma rows 64:128
                        nc.vector.reciprocal_approx_fast(out=rec, in_=acc[0])
                        nc.sync.dma_start(out=rec2[0:64, :], in_=rec[64:128, :])
                        nc.vector.tensor_mul(
                            out=on[0:64, :], in0=acc[0][0:64, :], in1=rec2[0:64, :]
                        )
                        # odd head: sigma rows 0:64, o rows 64:128
                        nc.vector.reciprocal_approx_fast(out=recb, in_=acc[1])
                        nc.sync.dma_start(out=rec2[64:128, :], in_=recb[0:64, :])
                        nc.vector.tensor_mul(
                            out=on[64:128, :], in0=acc[1][64:128, :],
                            in1=rec2[64:128, :],
                        )
                        on_sb[hp] = on
                    # proj for this chunk
                    for ot in range(CT):
                        pr = accp.tile([128, 512], f32, tag="acc", name="pr")
                        for hp in range(2):
                            nc.tensor.matmul(
                                pr,
                                pT_sb[hp][:, 128 * ot: 128 * (ot + 1)],
                                on_sb[hp],
                                start=(hp == 0), stop=(hp == 1),
                            )
                        out_t = outsb.tile([128, 512], f32, tag="out", name="out")
                        nc.vector.scalar_tensor_tensor(
                            out=out_t, in0=pr, scalar=pb_sb[:, ot: ot + 1],
                            in1=xq_sb[ot][:, ns], op0=Alu.add, op1=Alu.add,
                        )
                        nc.sync.dma_start(
                            out=y[128 * ot: 128 * (ot + 1), ns], in_=out_t
                        )

    if finalize:
        nc.finalize()
    else:
        nc.compile()
    return nc


_NC_CACHE = None


def _get_nc():
    global _NC_CACHE
    if _NC_CACHE is None:
        _NC_CACHE = _build_nc()
    return _NC_CACHE


def _make_in_maps(x, norm_w, norm_b, qkv_w, qkv_b, proj_w, proj_b):
    import ml_dtypes

    xr = np.ascontiguousarray(x.reshape(B, C, N), dtype=np.float32)
    wT = np.ascontiguousarray(qkv_w.astype(np.float32).T).astype(ml_dtypes.bfloat16)
    pT = np.ascontiguousarray(proj_w.astype(np.float32).T).astype(ml_dtypes.bfloat16)
    # v bias is not applied on device (attention weights sum to 1, so its
    # contribution to the output is proj_w @ v_bias, folded in here)
    proj_b = proj_b.astype(np.float32) + proj_w.astype(np.float32) @ qkv_b[
        2 * C: 3 * C].astype(np.float32)
    G = np.zeros((CT, 128, GROUPS), np.float32)
    P = np.zeros((CT, GROUPS, 128), np.float32)
    for ct in range(CT):
        for c in range(128):
            g = (128 * ct + c) // (C // GROUPS)
            G[ct, c, g] = 1.0 / (C // GROUPS)
            P[ct, g, c] = 1.0
    shared = {
        "wT": wT, "pT": pT,
        "qb": qkv_b.astype(np.float32), "pb": proj_b.astype(np.float32),
        "nw": norm_w.astype(np.float32), "nb": norm_b.astype(np.float32),
        "Gm": G, "Pm": P,
    }
    in_maps = []
    for core in range(NCORES):
        b = core // 4
        qs = (core % 4) * NQ
        m = dict(shared)
        m["xf"] = xr[b]
        m["xq"] = np.ascontiguousarray(xr[b][:, qs: qs + NQ])
        in_maps.append(m)
    return in_maps


def kernel(x, norm_w, norm_b, qkv_w, qkv_b, proj_w, proj_b, _trace=False):
    from concourse import bass_utils

    nc = _get_nc()
    in_maps = _make_in_maps(x, norm_w, norm_b, qkv_w, qkv_b, proj_w, proj_b)
    res = bass_utils.run_bass_kernel_spmd(
        nc, in_maps, core_ids=list(range(NCORES)), trace=_trace
    )
    out = np.empty((B, C, N), np.float32)
    for core in range(NCORES):
        b = core // 4
        qs = (core % 4) * NQ
        out[b][:, qs: qs + NQ] = res.results[core]["y"]
    out = out.reshape(B, C, 16, 16, 16)
    if _trace:
        return out, res
    return out



# revision 27
# speedup vs baseline: 1.3072x; 1.3072x over previous
"""AttentionBlock3D Trainium2 kernel.

Module: GroupNorm(8 groups) -> 1x1x1 conv QKV -> 4-head attention over
N=4096 spatial positions (head_dim 64) -> 1x1x1 conv proj -> residual.
Shapes: x [2, 256, 16, 16, 16] f32.

Sharding (8 cores): batch (2) x query-range (4 chunks of 1024 rows).
Each core computes, for its batch b and query rows nq:
  - GroupNorm stats over the full x[b] (redundant per-batch, cheap),
    folded into a per-channel affine (s_c, t_c) applied on the fly.
  - k, v for ALL 4096 keys (all heads), q only for its 1024 rows.
  - full attention for all 4 heads on its rows; softmax is computed
    unnormalized (exp, no max subtraction -- scores are O(1) here) with
    the denominator obtained by an ones-matmul on the PE, and the
    normalization folded in after the attention*V matmul.
  - proj + bias + residual for its [256, 1024] output slice.
Gather on host is pure concatenation.

Layouts on device (per core):
  x  [C=256, N]   -> 2 channel-tiles of [128, N] (channels on partitions)
  k_sb[pair]      [128, 4096] bf16: partitions = [head 2p (64) ; head 2p+1]
  q_sb[pair]      [128, 1024] bf16: same head-pair packing
  vT_sb           [128, 32*256] bf16: partitions = key-tile rows m,
                  free = m-tile * 256 + v-channel
  scores^T        PSUM [m 128, n 512] via row-tiled (K=64) matmul pairs
  attention out   o[c 128, n 512] via col-tiled (M=64) matmul pairs
Softmax denominator: ones[128,64] lhsT col-tiled -> sg[128,512] where
rows 0:64 = sigma(head even) replicated, 64:128 = sigma(head odd).
"""

import numpy as np

B = 2
C = 256
NH = 4
GROUPS = 8
EPS = 1e-5
N = 16 * 16 * 16  # 4096
HD = C // NH      # 64
NQ = N // 4       # 1024 query rows per core
NCORES = 8
CT = 2            # channel tiles of 128
MT = N // 128     # 32 key tiles
SCALE = HD ** -0.5
# Schraudolph bf16 exp: bits_i16(round(A*x + B)) viewed as bf16 ~= exp(x).
# A folds the attention scale; c tuned for min max-rel-err (~3.3%), which
# is benign here (scores span +-2.5, softmax-normalized, verified 1.3e-4
# end-to-end on this problem's data).
import math
EXP_A = SCALE * 128.0 / math.log(2.0)
EXP_B = 127.0 * 128.0 - 0.0430 * 128.0
# fp8e4m3 variant (int8 bits): [s][4 exp][3 mant]
EXP8_A = SCALE * 8.0 / math.log(2.0)
EXP8_B = 7.0 * 8.0 - 0.0430 * 8.0


def _build_nc(finalize=True):
    import concourse.bacc as bacc
    import concourse.bass as bass
    import concourse.mybir as mybir
    from concourse.tile import TileContext

    f32 = mybir.dt.float32
    bf16 = mybir.dt.bfloat16
    Alu = mybir.AluOpType
    AF = mybir.ActivationFunctionType

    nc = bacc.Bacc("TRN2", debug=False)

    xf = nc.dram_tensor("xf", [C, N], f32, kind="ExternalInput").ap()
    xq = nc.dram_tensor("xq", [C, NQ], f32, kind="ExternalInput").ap()
    wT = nc.dram_tensor("wT", [C, 3 * C], bf16, kind="ExternalInput").ap()
    pT = nc.dram_tensor("pT", [C, C], bf16, kind="ExternalInput").ap()
    qb = nc.dram_tensor("qb", [3 * C], f32, kind="ExternalInput").ap()
    pb = nc.dram_tensor("pb", [C], f32, kind="ExternalInput").ap()
    nw = nc.dram_tensor("nw", [C], f32, kind="ExternalInput").ap()
    nb = nc.dram_tensor("nb", [C], f32, kind="ExternalInput").ap()
    Gm = nc.dram_tensor("Gm", [CT, 128, GROUPS], f32, kind="ExternalInput").ap()
    Pm = nc.dram_tensor("Pm", [CT, GROUPS, 128], f32, kind="ExternalInput").ap()
    y = nc.dram_tensor("y", [C, NQ], f32, kind="ExternalOutput").ap()

    def bcast_ap(src_1d, parts):
        # view a 1-D dram AP as [parts, len] with 0-stride partition dim
        return bass.AP(
            tensor=src_1d.tensor,
            offset=src_1d.offset,
            ap=[[0, parts]] + list(src_1d.ap),
        )

    with TileContext(nc) as tc:
        import contextlib

        est = contextlib.ExitStack()
        with est:
            singles = est.enter_context(tc.tile_pool(name="singles", bufs=1))
            work = est.enter_context(tc.tile_pool(name="work", bufs=2))

            # ---------- persistent SBUF tiles ----------
            x_sb = [singles.tile([128, N], f32, tag=f"x{ct}", name=f"x{ct}") for ct in range(CT)]
            xq_sb = [singles.tile([128, NQ], f32, tag=f"xq{ct}", name=f"xq{ct}") for ct in range(CT)]
            xn_sb = [singles.tile([128, N], bf16, tag=f"xn{ct}", name=f"xn{ct}") for ct in range(CT)]
            xqn_sb = [singles.tile([128, NQ], bf16, tag=f"xqn{ct}", name=f"xqn{ct}") for ct in range(CT)]
            wT_sb = [singles.tile([128, 3 * C], bf16, tag=f"wT{ct}", name=f"wT{ct}") for ct in range(CT)]
            pT_sb = [singles.tile([128, C], bf16, tag=f"pT{ct}", name=f"pT{ct}") for ct in range(CT)]
            k_sb = [singles.tile([128, N], bf16, tag=f"k{p}", name=f"k{p}") for p in range(2)]
            q_sb = [singles.tile([128, NQ], bf16, tag=f"q{p}", name=f"q{p}") for p in range(2)]
            # vT2: bf16 AV stationary operand.
            # [p, mt(32):512, hp(2):256, hh(2):128, d(128):1]
            # where d = [v(64)|ones(64)] for hh=0, [ones(64)|v(64)] for hh=1.
            vT2_sb = singles.tile([128, MT * 512], bf16, tag="vT2", name="vT2")
            qb_sb = singles.tile([128, 6], f32, tag="qb", name="qb")
            pb_sb = singles.tile([128, 2], f32, tag="pb", name="pb")
            nw_sb = singles.tile([128, CT], f32, tag="nw", name="nw")
            nb_sb = singles.tile([128, CT], f32, tag="nb", name="nb")
            G_sb = [singles.tile([128, GROUPS], f32, tag=f"G{ct}", name=f"G{ct}") for ct in range(CT)]
            P_sb = [singles.tile([GROUPS, 128], f32, tag=f"P{ct}", name=f"P{ct}") for ct in range(CT)]
            eps_sb = singles.tile([128, 1], f32, tag="eps", name="eps")
            s_sb = [singles.tile([128, 1], f32, tag=f"s{ct}", name=f"s{ct}") for ct in range(CT)]
            t_sb = [singles.tile([128, 1], f32, tag=f"t{ct}", name=f"t{ct}") for ct in range(CT)]
            mr_sb = singles.tile([GROUPS, 2], f32, tag="mr", name="mr")

            # ---------- input DMA ----------
            for ct in range(CT):
                cs = slice(128 * ct, 128 * (ct + 1))
                for dc in range(4):
                    ds_ = slice(1024 * dc, 1024 * (dc + 1))
                    nc.sync.dma_start(out=x_sb[ct][:, ds_], in_=xf[cs, ds_])
                nc.sync.dma_start(out=xq_sb[ct], in_=xq[cs, :])
                nc.sync.dma_start(out=wT_sb[ct], in_=wT[cs, :])
                nc.sync.dma_start(out=pT_sb[ct], in_=pT[cs, :])
                nc.sync.dma_start(out=G_sb[ct], in_=Gm[ct])
                nc.sync.dma_start(out=P_sb[ct], in_=Pm[ct])
            nc.sync.dma_start(out=qb_sb, in_=qb.rearrange("(t p) -> p t", p=128))
            nc.sync.dma_start(out=pb_sb, in_=pb.rearrange("(t p) -> p t", p=128))
            nc.sync.dma_start(out=nw_sb, in_=nw.rearrange("(t p) -> p t", p=128))
            nc.sync.dma_start(out=nb_sb, in_=nb.rearrange("(t p) -> p t", p=128))
            nc.vector.memset(eps_sb, EPS)

            # ---------- GroupNorm statistics ----------
            with tc.tile_pool(name="ph1psum", bufs=1, space="PSUM") as pp, \
                 tc.tile_pool(name="warm", bufs=1, space="PSUM") as wmp, \
                 tc.tile_pool(name="stats", bufs=2) as stp:
                # modest PE pre-warm while the stats chain runs, so the QKV
                # matmuls start at 2.4 GHz instead of the throttled clock
                warm_ps = wmp.tile([128, 512], f32, name="warm_ps")
                for _ in range(18):
                    nc.tensor.matmul(
                        warm_ps,
                        x_sb[0][:, 0:128],
                        x_sb[0][:, 0:512],
                        start=True, stop=True,
                    )
                gs_ps = pp.tile([GROUPS, 2], f32, tag="gs", name="gs")
                NSUB = N // 512
                for ct in range(CT):
                    stats = stp.tile([128, NSUB, 6], f32, tag="bnst", name="bnst")
                    for i in range(NSUB):
                        nc.vector.bn_stats(
                            out=stats[:, i, :], in_=x_sb[ct][:, 512 * i: 512 * (i + 1)]
                        )
                    mv = stp.tile([128, 2], f32, tag="mv", name="mv")
                    nc.vector.bn_aggr(out=mv, in_=stats)
                    # cstat = [mean_c, E[x^2]_c]
                    cstat = stp.tile([128, 2], f32, tag="cstat", name="cstat")
                    nc.vector.tensor_copy(out=cstat[:, 0:1], in_=mv[:, 0:1])
                    m2 = stp.tile([128, 1], f32, tag="m2", name="m2")
                    nc.vector.tensor_mul(out=m2, in0=mv[:, 0:1], in1=mv[:, 0:1])
                    nc.vector.tensor_add(out=cstat[:, 1:2], in0=mv[:, 1:2], in1=m2)
                    # group sums: gs[g, :] = sum_c G[c, g] * cstat[c, :] / (fold 1/32 in G)
                    nc.tensor.matmul(
                        gs_ps, G_sb[ct], cstat, start=(ct == 0), stop=(ct == CT - 1)
                    )
                # mr = [mean_g, rstd_g]
                nc.vector.tensor_copy(out=mr_sb[:, 0:1], in_=gs_ps[:, 0:1])
                gm2 = stp.tile([GROUPS, 1], f32, tag="gm2", name="gm2")
                # only one DVE input may come from PSUM -> square the SBUF copy
                nc.vector.tensor_mul(out=gm2, in0=mr_sb[:, 0:1], in1=mr_sb[:, 0:1])
                var_g = stp.tile([GROUPS, 1], f32, tag="varg", name="varg")
                nc.vector.tensor_sub(out=var_g, in0=gs_ps[:, 1:2], in1=gm2)
                sd_g = stp.tile([GROUPS, 1], f32, tag="sdg", name="sdg")
                nc.scalar.activation(
                    out=sd_g, in_=var_g, func=AF.Sqrt, bias=eps_sb[0:GROUPS, :],
                )
                nc.vector.reciprocal(out=mr_sb[:, 1:2], in_=sd_g)
                # broadcast to channels, build per-channel affine s, t
                for ct in range(CT):
                    pc_ps = pp.tile([128, 2], f32, tag="pc", name="pc")
                    nc.tensor.matmul(pc_ps, P_sb[ct], mr_sb, start=True, stop=True)
                    nc.vector.tensor_mul(
                        out=s_sb[ct], in0=pc_ps[:, 1:2], in1=nw_sb[:, ct: ct + 1]
                    )
                    tt = stp.tile([128, 1], f32, tag="tt", name="tt")
                    nc.vector.tensor_mul(out=tt, in0=pc_ps[:, 0:1], in1=s_sb[ct])
                    nc.vector.tensor_sub(
                        out=t_sb[ct], in0=nb_sb[:, ct: ct + 1], in1=tt
                    )
                # normalized inputs (bf16): xn = x * s + t  (on ACT, per-partition affine)
                for ct in range(CT):
                    for dc in range(4):
                        ds_ = slice(1024 * dc, 1024 * (dc + 1))
                        nc.vector.tensor_scalar(
                            out=xn_sb[ct][:, ds_], in0=x_sb[ct][:, ds_],
                            scalar1=s_sb[ct], scalar2=t_sb[ct],
                            op0=Alu.mult, op1=Alu.add,
                        )
                    nc.vector.tensor_scalar(
                        out=xqn_sb[ct], in0=xq_sb[ct], scalar1=s_sb[ct],
                        scalar2=t_sb[ct], op0=Alu.mult, op1=Alu.add,
                    )

            # ---------- QKV projections ----------
            # vT8 holds fp8 v (no bias: it is folded into the proj bias on
            # the host, valid because the attention weights sum to 1) plus
            # ones columns that make each AV matmul also produce the softmax
            # denominator in the complementary partition rows of the same
            # accumulator bank, for free.
            def v2_view(off, dims):
                return bass.AP(
                    tensor=vT2_sb.tensor,
                    offset=vT2_sb.offset + off,
                    ap=[list(vT2_sb.ap[0])] + [list(d) for d in dims],
                )

            # ones blocks: hh=0 -> block d 64:128, hh=1 -> block d 0:64
            for hh in range(2):
                nc.gpsimd.memset(
                    v2_view(64 * (1 + hh),
                            [[512, MT], [256, 2], [1, 64]]),
                    1.0,
                )
            with tc.tile_pool(name="qkvpsum", bufs=3, space="PSUM") as qp, \
                 tc.tile_pool(name="vtpsum", bufs=3, space="PSUM") as vp:
                # q first: unblocks the first attention iteration earliest
                for hp in range(2):
                    for chk in range(NQ // 512):
                        ns = slice(512 * chk, 512 * (chk + 1))
                        qps = qp.tile([128, 512], f32, tag="kq", name="kq")
                        for ct in range(CT):
                            nc.tensor.matmul(
                                qps,
                                wT_sb[ct][:, 128 * hp: 128 * (hp + 1)],
                                xqn_sb[ct][:, ns],
                                start=(ct == 0), stop=(ct == CT - 1),
                            )
                        nc.scalar.activation(
                            out=q_sb[hp][:, ns], in_=qps, func=AF.Identity,
                            bias=qb_sb[:, hp: hp + 1],
                        )
                # k for pair 0, then vT (attention on pair 0 needs both),
                # then k for pair 1.
                def emit_k(hp):
                    for chk in range(N // 512):
                        ns = slice(512 * chk, 512 * (chk + 1))
                        kp = qp.tile([128, 512], f32, tag="kq", name="kq")
                        for ct in range(CT):
                            nc.tensor.matmul(
                                kp,
                                wT_sb[ct][:, C + 128 * hp: C + 128 * (hp + 1)],
                                xn_sb[ct][:, ns],
                                start=(ct == 0), stop=(ct == CT - 1),
                            )
                        nc.scalar.activation(
                            out=k_sb[hp][:, ns], in_=kp, func=AF.Identity,
                            bias=qb_sb[:, 2 + hp: 3 + hp],
                        )

                emit_k(0)
                for mtp in range(MT // 2):
                    vps = vp.tile([128, 2 * C], f32, tag="vt", name="vt")
                    for j in range(2):
                        ms = slice(128 * (2 * mtp + j), 128 * (2 * mtp + j + 1))
                        for ct in range(CT):
                            nc.tensor.matmul(
                                vps[:, C * j: C * (j + 1)],
                                xn_sb[ct][:, ms],
                                wT_sb[ct][:, 2 * C: 3 * C],
                                start=(ct == 0), stop=(ct == CT - 1),
                            )
                    # strided f32->bf16 converts scattering the v channels
                    # into the vT2 layout (one per head pair: ISA caps APs at
                    # 3 free dims); v channel c = (hp, hh, dv), d-off 64*hh.
                    src = vps.rearrange(
                        "p (j hp hh dv) -> p hp j hh dv", j=2, hp=2, hh=2
                    )
                    for hp_ in range(2):
                        dst = v2_view(
                            1024 * mtp + 256 * hp_,
                            [[512, 2], [192, 2], [1, 64]],
                        )
                        if (2 * mtp + hp_) % 2 == 0:
                            nc.scalar.copy(out=dst, in_=src[:, hp_])
                        else:
                            nc.vector.tensor_copy(out=dst, in_=src[:, hp_])
                emit_k(1)

            # ---------- attention + proj ----------
            # PSUM budget (8 banks): scores pool [128,1536]x2 = 6 banks,
            # "acc" tag pool [128,512]x2 = 2 banks. Each accumulator bank
            # receives one combined AV+sigma matmul stream (full M=128:
            # 64 v columns + 64 ones columns), so o and its softmax
            # denominator land in complementary partition halves of the
            # same bank. The proj matmuls reuse the acc slots.
            GRPS = [list(range(i, min(i + 2, MT))) for i in range(0, MT, 2)]
            with tc.tile_pool(name="scps", bufs=3, space="PSUM") as scp, \
                 tc.tile_pool(name="accps", bufs=2, space="PSUM") as accp, \
                 tc.tile_pool(name="esb", bufs=3) as esb, \
                 tc.tile_pool(name="osb", bufs=2) as osb, \
                 tc.tile_pool(name="outsb", bufs=2) as outsb:
                for cn in range(NQ // 512):
                    ns = slice(512 * cn, 512 * (cn + 1))
                    on_sb = [None, None]
                    for hp in range(2):
                        # acc[hh]: even head: [o(0:64); sigma(64:128)]
                        #          odd head:  [sigma(0:64); o(64:128)]
                        acc = [accp.tile([128, 512], f32, tag="acc", name=f"acc{h}")
                               for h in range(2)]
                        for gi, grp in enumerate(GRPS):
                            gl = len(grp)
                            e_h = []
                            for hh in range(2):
                                sc = scp.tile([128, 1024], f32, tag="sc", name="sc")
                                for j, mt in enumerate(grp):
                                    nc.tensor.matmul(
                                        sc[:, 512 * j: 512 * (j + 1)],
                                        k_sb[hp][64 * hh: 64 * (hh + 1),
                                                 128 * mt: 128 * (mt + 1)],
                                        q_sb[hp][64 * hh: 64 * (hh + 1), ns],
                                        start=True, stop=True,
                                        tile_position=(64 * hh, 0),
                                    )
                                e = esb.tile([128, 1024], bf16, tag="e", name="e")
                                # ACT does exact exp; DVE does Schraudolph
                                # (int16 bits of the target bf16). hh0 -> ACT,
                                # hh1 -> DVE except 1/16 groups (balance).
                                on_act = (hh == 0) or (gi % 16 == 15)
                                if on_act:
                                    nc.scalar.activation(
                                        out=e[:, : 512 * gl], in_=sc[:, : 512 * gl],
                                        func=AF.Exp, scale=SCALE,
                                    )
                                else:
                                    nc.vector.tensor_scalar(
                                        out=e.bitcast(mybir.dt.int16)[:, : 512 * gl],
                                        in0=sc[:, : 512 * gl],
                                        scalar1=EXP_A, scalar2=EXP_B,
                                        op0=Alu.mult, op1=Alu.add,
                                    )
                                e_h.append(e)
                            for j, mt in enumerate(grp):
                                ej = slice(512 * j, 512 * (j + 1))
                                first = (gi == 0 and j == 0)
                                last = (gi == len(GRPS) - 1 and j == gl - 1)
                                for hh in range(2):
                                    nc.tensor.matmul(
                                        acc[hh],
                                        vT2_sb[:, 512 * mt + 256 * hp + 128 * hh:
                                               512 * mt + 256 * hp + 128 * (hh + 1)],
                                        e_h[hh][:, ej],
                                        start=first, stop=last,
                                    )
                        # normalize: sigma sits in the complementary partition
                        # half; reciprocal locally, DMA the [64, 512] block to
                        # the o rows' partitions, multiply.
                        rec = osb.tile([128, 512], f32, tag="rec", name="rec")
                        recb = osb.tile([128, 512], f32, tag="recb", name="recb")
                        rec2 = osb.tile([128, 512], f32, tag="rec2", name="rec2")
                        on = osb.tile([128, 512], bf16, tag="on", name="on")
                        # reciprocal_approx_fast requires base_partition 0, so
                        # run it over the full tile; only the sigma half of
                        # each result is used (the o half is don't-care).
                        # even head: o rows 0:64, sigma rows 64:128
                        nc.vector.reciprocal_approx_fast(out=rec, in_=acc[0])
                        nc.sync.dma_start(out=rec2[0:64, :], in_=rec[64:128, :])
                        nc.vector.tensor_mul(
                            out=on[0:64, :], in0=acc[0][0:64, :], in1=rec2[0:64, :]
                        )
                        # odd head: sigma rows 0:64, o rows 64:128
                        nc.vector.reciprocal_approx_fast(out=recb, in_=acc[1])
                        nc.sync.dma_start(out=rec2[64:128, :], in_=recb[0:64, :])
                        nc.vector.tensor_mul(
                            out=on[64:128, :], in0=acc[1][64:128, :],
                            in1=rec2[64:128, :],
                        )
                        on_sb[hp] = on
                    # proj for this chunk
                    for ot in range(CT):
                        pr = accp.tile([128, 512], f32, tag="acc", name="pr")
                        for hp in range(2):
                            nc.tensor.matmul(
                                pr,
                                pT_sb[hp][:, 128 * ot: 128 * (ot + 1)],
                                on_sb[hp],
                                start=(hp == 0), stop=(hp == 1),
                            )
                        out_t = outsb.tile([128, 512], f32, tag="out", name="out")
                        nc.vector.scalar_tensor_tensor(
                            out=out_t, in0=pr, scalar=pb_sb[:, ot: ot + 1],
                            in1=xq_sb[ot][:, ns], op0=Alu.add, op1=Alu.add,
                        )
                        nc.sync.dma_start(
                            out=y[128 * ot: 128 * (ot + 1), ns], in_=out_t
                        )

    if finalize:
        nc.finalize()
    else:
        nc.compile()
    return nc


_NC_CACHE = None


def _get_nc():
    global _NC_CACHE
    if _NC_CACHE is None:
        _NC_CACHE = _build_nc()
    return _NC_CACHE


def _make_in_maps(x, norm_w, norm_b, qkv_w, qkv_b, proj_w, proj_b):
    import ml_dtypes

    xr = np.ascontiguousarray(x.reshape(B, C, N), dtype=np.float32)
    wT = np.ascontiguousarray(qkv_w.astype(np.float32).T).astype(ml_dtypes.bfloat16)
    pT = np.ascontiguousarray(proj_w.astype(np.float32).T).astype(ml_dtypes.bfloat16)
    # v bias is not applied on device (attention weights sum to 1, so its
    # contribution to the output is proj_w @ v_bias, folded in here)
    proj_b = proj_b.astype(np.float32) + proj_w.astype(np.float32) @ qkv_b[
        2 * C: 3 * C].astype(np.float32)
    G = np.zeros((CT, 128, GROUPS), np.float32)
    P = np.zeros((CT, GROUPS, 128), np.float32)
    for ct in range(CT):
        for c in range(128):
            g = (128 * ct + c) // (C // GROUPS)
            G[ct, c, g] = 1.0 / (C // GROUPS)
            P[ct, g, c] = 1.0
    shared = {
        "wT": wT, "pT": pT,
        "qb": qkv_b.astype(np.float32), "pb": proj_b.astype(np.float32),
        "nw": norm_w.astype(np.float32), "nb": norm_b.astype(np.float32),
        "Gm": G, "Pm": P,
    }
    in_maps = []
    for core in range(NCORES):
        b = core // 4
        qs = (core % 4) * NQ
        m = dict(shared)
        m["xf"] = xr[b]
        m["xq"] = np.ascontiguousarray(xr[b][:, qs: qs + NQ])
        in_maps.append(m)
    return in_maps


def kernel(x, norm_w, norm_b, qkv_w, qkv_b, proj_w, proj_b, _trace=False):
    from concourse import bass_utils

    nc = _get_nc()
    in_maps = _make_in_maps(x, norm_w, norm_b, qkv_w, qkv_b, proj_w, proj_b)
    res = bass_utils.run_bass_kernel_spmd(
        nc, in_maps, core_ids=list(range(NCORES)), trace=_trace
    )
    out = np.empty((B, C, N), np.float32)
    for core in range(NCORES):
        b = core // 4
        qs = (core % 4) * NQ
        out[b][:, qs: qs + NQ] = res.results[core]["y"]
    out = out.reshape(B, C, 16, 16, 16)
    if _trace:
        return out, res
    return out

